# revision 40
# baseline (speedup 1.0000x reference)
import os
import sys
import numpy as np

sys.path.insert(0, "/opt/trn_rl_repo")

from contextlib import ExitStack

from concourse import bacc, bass, bass_utils, mybir, tile

# Problem dims (hardcoded per spec)
B, T, D = 4, 256, 512
L, H = 2, 8
I = 2 * D          # 1024
DH = I // H        # 128
K, G = 32, 64
EPS = 1e-5
NKC = I // 128     # 8
DC = D // 128      # 4
MT = K + 2 * K * G  # 32 + 4096 = 4128 (pi | mu | sigma logits)

FP = mybir.dt.float32
FR = mybir.dt.float32r
Act = mybir.ActivationFunctionType
Alu = mybir.AluOpType

NEG = -30000.0


def _r(ap):
    return ap.bitcast(FR)


def build_nc():
    nc = bacc.Bacc()
    d = {}

    def din(name, shape):
        d[name] = nc.declare_dram_parameter(name, list(shape), FP, False)

    din("xT", (D, T))
    din("wproj", (L * D, 2 * I))
    din("wq", (L * I, I))
    din("wk", (L * I, I))
    din("wv", (L * I, I))
    din("wi", (L * I, H))
    din("wf", (L * I, H))
    din("bib", (L * H, 1))
    din("bfn", (L * H, 1))
    din("lgr", (L * 2 * DC, 128))
    din("lnsc", (L * 128, DC))
    din("ggr", (L * 2 * H, 128))
    din("gnc", (L * 128, H))
    din("wdown", (L * I, D))
    din("ltb0", (128, T))
    din("ltb1", (128, T))
    din("ident", (128, 128))
    din("rsel", (H, H * 128))
    din("flns", (128, DC))
    din("flnb", (128, DC))
    din("wcat", (D, MT))
    d["mall"] = nc.declare_dram_parameter("mall", [1, MT], FP, True)

    with ExitStack() as es:
        tc = es.enter_context(tile.TileContext(nc))
        const = es.enter_context(tc.tile_pool(name="const", bufs=1))
        wts = es.enter_context(tc.tile_pool(name="wts", bufs=1))
        acts = es.enter_context(tc.tile_pool(name="acts", bufs=1))
        rows = es.enter_context(tc.tile_pool(name="rows", bufs=1))
        stream = es.enter_context(tc.tile_pool(name="stream", bufs=1))
        pp = es.enter_context(tc.tile_pool(name="pp", bufs=1, space="PSUM"))

        dma = nc.sync.dma_start
        mm = nc.tensor.matmul
        act = nc.scalar.activation
        vec = nc.vector

        # constants
        idn = const.tile((128, 128), FP, tag="ident")
        dma(out=_r(idn), in_=_r(d["ident"][:, :]))
        ltb = []
        for i in (0, 1):
            t = const.tile((128, T), FP, tag=f"ltb{i}")
            dma(out=_r(t), in_=_r(d[f"ltb{i}"][:, :]))
            ltb.append(t)
        onescol = const.tile((128, 1), FP, tag="onescol")
        vec.memset(onescol, 1.0)
        onesrow = const.tile((1, 128), FP, tag="onesrow")
        vec.memset(onesrow, 1.0)
        epsc = const.tile((128, 1), FP, tag="epsc")
        vec.memset(epsc, EPS)
        epsb = epsc[0:1, 0:1]
        rsel = const.tile((H, H * 128), FP, tag="rsel")
        dma(out=rsel, in_=d["rsel"][:, :])
        flns = const.tile((128, DC), FP, tag="flns")
        dma(out=flns, in_=d["flns"][:, :])
        flnb = const.tile((128, DC), FP, tag="flnb")
        dma(out=flnb, in_=d["flnb"][:, :])
        ones1T = const.tile((1, T), FP, tag="ones1T")
        vec.memset(ones1T, 1.0)

        # h^T in 4 chunks of (128, T)
        hT = []
        for dc in range(DC):
            t = acts.tile((128, T), FP, tag=f"hT{dc}", bufs=2)
            dma(out=t, in_=d["xT"][dc * 128:(dc + 1) * 128, :])
            hT.append(t)

        for l in range(L):
            # per-layer small weights
            lgs, lgb = [], []
            for dc in range(DC):
                ts = wts.tile((1, 128), FP, tag=f"lgs{dc}", bufs=2)
                dma(out=ts, in_=d["lgr"][l * 2 * DC + 2 * dc:l * 2 * DC + 2 * dc + 1, :])
                lgs.append(ts)
                tb = wts.tile((1, 128), FP, tag=f"lgb{dc}", bufs=2)
                dma(out=tb,
                    in_=d["lgr"][l * 2 * DC + 2 * dc + 1:l * 2 * DC + 2 * dc + 2, :])
                lgb.append(tb)
            lnsc = wts.tile((128, DC), FP, tag="lnsc", bufs=2)
            dma(out=lnsc, in_=d["lnsc"][l * 128:(l + 1) * 128, :])
            ggs, ggb = [], []
            for h in range(H):
                ts = wts.tile((1, 128), FP, tag=f"ggs{h}", bufs=2)
                dma(out=ts, in_=d["ggr"][l * 2 * H + 2 * h:l * 2 * H + 2 * h + 1, :])
                ggs.append(ts)
                tb = wts.tile((1, 128), FP, tag=f"ggb{h}", bufs=2)
                dma(out=tb,
                    in_=d["ggr"][l * 2 * H + 2 * h + 1:l * 2 * H + 2 * h + 2, :])
                ggb.append(tb)
            gnc = wts.tile((128, H), FP, tag="gnc", bufs=2)
            dma(out=gnc, in_=d["gnc"][l * 128:(l + 1) * 128, :])
            bibt = wts.tile((H, 1), FP, tag="bib", bufs=2)
            dma(out=bibt, in_=d["bib"][l * H:(l + 1) * H, :])
            bfnt = wts.tile((H, 1), FP, tag="bfn", bufs=2)
            dma(out=bfnt, in_=d["bfn"][l * H:(l + 1) * H, :])
            wit, wft = [], []
            for kc in range(NKC):
                ti = wts.tile((128, H), FP, tag=f"wi{kc}", bufs=2)
                dma(out=_r(ti), in_=_r(d["wi"][l * I + kc * 128:l * I + (kc + 1) * 128, :]))
                wit.append(ti)
                tf = wts.tile((128, H), FP, tag=f"wf{kc}", bufs=2)
                dma(out=_r(tf), in_=_r(d["wf"][l * I + kc * 128:l * I + (kc + 1) * 128, :]))
                wft.append(tf)

            # ---- Phase A: LayerNorm over D (features on partitions) ----
            psum_s = pp.tile((1, T), FP, tag="prow", bufs=2)
            for dc in range(DC):
                mm(psum_s, onescol, hT[dc], start=(dc == 0), stop=(dc == DC - 1))
            psum_q = pp.tile((1, T), FP, tag="prow", bufs=2)
            for dc in range(DC):
                sq = acts.tile((128, T), FP, tag="sq", bufs=2)
                act(sq, hT[dc], Act.Square)
                mm(psum_q, onescol, sq, start=(dc == 0), stop=(dc == DC - 1))
            mean = rows.tile((1, T), FP, tag="mean", bufs=2)
            vec.tensor_scalar_mul(mean, psum_s, 1.0 / D)
            ex2 = rows.tile((1, T), FP, tag="ex2", bufs=2)
            vec.tensor_scalar_mul(ex2, psum_q, 1.0 / D)
            msq = rows.tile((1, T), FP, tag="msq", bufs=2)
            act(msq, mean, Act.Square)
            var = rows.tile((1, T), FP, tag="var", bufs=2)
            vec.tensor_tensor(var, ex2, msq, Alu.subtract)
            std = rows.tile((1, T), FP, tag="std", bufs=2)
            act(std, var, Act.Sqrt, bias=epsb)
            rstd = rows.tile((1, T), FP, tag="rstd", bufs=2)
            vec.reciprocal(rstd, std)
            negmr = rows.tile((1, T), FP, tag="two", bufs=2)
            vec.scalar_tensor_tensor(negmr, mean, -1.0, rstd, Alu.mult, Alu.mult)
            pRb = pp.tile((128, T), FP, tag="pbN", bufs=2)
            mm(pRb, onesrow, rstd, start=True, stop=True)
            xn = []
            for dc in range(DC):
                pBG = pp.tile((128, T), FP, tag="pb", bufs=2)
                mm(pBG, lgs[dc], negmr, start=True, stop=False)
                mm(pBG, lgb[dc], ones1T, start=False, stop=True)
                tmp = acts.tile((128, T), FP, tag="lntmp", bufs=2)
                vec.tensor_tensor(tmp, hT[dc], pRb, Alu.mult)
                xt = acts.tile((128, T), FP, tag=f"xn{dc}", bufs=2)
                vec.scalar_tensor_tensor(_r(xt), tmp, lnsc[:, dc:dc + 1], pBG,
                                         Alu.mult, Alu.add)
                xn.append(xt)

            # ---- Phase B: x @ W_proj -> xm (8 chunks), silu(z) (8 chunks) ----
            xm, sz = [], []
            for m in range(16):
                pm = pp.tile((128, T), FP, tag="pb", bufs=2)
                for kc in range(DC):
                    wpt = stream.tile((128, 128), FP, tag="wp", bufs=4)
                    dma(out=_r(wpt),
                        in_=_r(d["wproj"][l * D + kc * 128:l * D + (kc + 1) * 128,
                                          m * 128:(m + 1) * 128]))
                    mm(pm, _r(wpt), _r(xn[kc]), start=(kc == 0), stop=(kc == DC - 1))
                if m < 8:
                    t = acts.tile((128, T), FP, tag=f"xm{m}", bufs=2)
                    act(_r(t), pm, Act.Copy)
                    xm.append(t)
                else:
                    t = acts.tile((128, T), FP, tag=f"sz{m - 8}", bufs=2)
                    act(t, pm, Act.Silu)
                    sz.append(t)

            # ---- Phase C: gates ----
            pgi = pp.tile((H, T), FP, tag="prow", bufs=2)
            pgf = pp.tile((H, T), FP, tag="prow", bufs=2)
            for kc in range(NKC):
                mm(pgi, _r(wit[kc]), _r(xm[kc]), start=(kc == 0), stop=(kc == NKC - 1))
                mm(pgf, _r(wft[kc]), _r(xm[kc]), start=(kc == 0), stop=(kc == NKC - 1))
            enf = rows.tile((H, T), FP, tag="enf", bufs=2)
            act(enf, pgf, Act.Exp, bias=bfnt, scale=-1.0)
            sp = rows.tile((H, T), FP, tag="sp", bufs=2)
            act(sp, enf, Act.Ln, bias=1.0)
            ipb = rows.tile((H, T), FP, tag="ipb", bufs=2)
            act(ipb, pgi, Act.Identity, bias=bibt)
            Gt = rows.tile((H, T), FP, tag="Gt", bufs=2)
            vec.tensor_tensor_scan(Gt, sp, sp, 0.0, Alu.add, Alu.bypass)
            At = rows.tile((H, T), FP, tag="At", bufs=2)
            vec.tensor_tensor(At, ipb, Gt, Alu.add)
            rt = rows.tile((H, T), FP, tag="rt", bufs=2)
            vec.tensor_tensor_scan(rt, At, At, 0.0, Alu.max, Alu.bypass)
            negR = rows.tile((H, T), FP, tag="negR", bufs=2)
            vec.tensor_scalar_mul(negR, rt, -1.0)
            aT = []
            for sc in range(2):
                pt = pp.tile((128, H), FP, tag="prow", bufs=2)
                nc.tensor.transpose(pt, At[:, sc * 128:(sc + 1) * 128], idn[0:H, 0:H])
                t = acts.tile((128, H), FP, tag=f"aT{sc}", bufs=2)
                act(t, pt, Act.Copy)
                aT.append(t)

            # ---- Phase D: V (rows = s, cols = heads*dh), 2x2 chunks ----
            V4 = {}
            for g in range(2):
                pv = [pp.tile((128, 512), FP, tag="pvg", bufs=2, name=f"pv{_i}")
                      for _i in range(2)]
                for kc in range(NKC):
                    wvt = stream.tile((128, 512), FP, tag="wv", bufs=3)
                    dma(out=_r(wvt),
                        in_=_r(d["wv"][l * I + kc * 128:l * I + (kc + 1) * 128,
                                       g * 512:(g + 1) * 512]))
                    for sc in range(2):
                        mm(pv[sc], _r(xm[kc][:, sc * 128:(sc + 1) * 128]), _r(wvt),
                           start=(kc == 0), stop=(kc == NKC - 1))
                for sc in range(2):
                    t = acts.tile((128, 512), FP, tag=f"V{g}{sc}", bufs=2)
                    act(_r(t), pv[sc], Act.Copy)
                    V4[(g, sc)] = t

            # ---- Phase E: per-head attention + head LN + gate ----
            outT = []
            for h in range(H):
                g, hc = h // 4, h % 4
                pq = pp.tile((128, T), FP, tag="pb", bufs=2)
                for kc in range(NKC):
                    wqt = stream.tile((128, 128), FP, tag="wqk", bufs=6)
                    dma(out=_r(wqt),
                        in_=_r(d["wq"][l * I + kc * 128:l * I + (kc + 1) * 128,
                                       h * 128:(h + 1) * 128]))
                    mm(pq, _r(wqt), _r(xm[kc]), start=(kc == 0), stop=(kc == NKC - 1))
                qTs = acts.tile((128, T), FP, tag="qT", bufs=2)
                act(_r(qTs), pq, Act.Copy)
                pk = pp.tile((128, T), FP, tag="pb", bufs=2)
                for kc in range(NKC):
                    wkt = stream.tile((128, 128), FP, tag="wqk", bufs=6)
                    dma(out=_r(wkt),
                        in_=_r(d["wk"][l * I + kc * 128:l * I + (kc + 1) * 128,
                                       h * 128:(h + 1) * 128]))
                    mm(pk, _r(wkt), _r(xm[kc]), start=(kc == 0), stop=(kc == NKC - 1))
                kTs = acts.tile((128, T), FP, tag="kT", bufs=2)
                act(_r(kTs), pk, Act.Copy, scale=float(1.0 / np.sqrt(DH)))

                pnum = pp.tile((128, T), FP, tag="pbN", bufs=2)
                pden = pp.tile((1, T), FP, tag="prow", bufs=2)
                for sc in range(2):
                    pS = pp.tile((128, T), FP, tag="pb", bufs=2)
                    mm(pS, _r(kTs[:, sc * 128:(sc + 1) * 128]), _r(qTs),
                       start=True, stop=True)
                    pE = pp.tile((128, T), FP, tag="pb", bufs=2)
                    mm(pE, rsel[:, h * 128:(h + 1) * 128], negR, start=True, stop=False)
                    mm(pE, _r(idn), _r(ltb[sc]), start=False, stop=True)
                    Dx = acts.tile((128, T), FP, tag="Dx", bufs=3)
                    act(Dx, pE, Act.Exp, bias=aT[sc][:, h:h + 1])
                    SDT = acts.tile((128, T), FP, tag=f"SD{sc}", bufs=2)
                    vec.tensor_tensor(_r(SDT), Dx, pS, Alu.mult)
                    mm(pnum, _r(V4[(g, sc)][:, hc * 128:(hc + 1) * 128]), _r(SDT),
                       start=(sc == 0), stop=(sc == 1))
                    mm(pden, onescol, SDT, start=(sc == 0), stop=(sc == 1))
                nums = acts.tile((128, T), FP, tag="nums", bufs=2)
                act(nums, pnum, Act.Copy)
                sqs = acts.tile((128, T), FP, tag="sqs", bufs=2)
                act(sqs, pnum, Act.Square)
                pmu = pp.tile((1, T), FP, tag="prow", bufs=2)
                mm(pmu, onescol, nums, start=True, stop=True)
                psq = pp.tile((1, T), FP, tag="prow", bufs=2)
                mm(psq, onescol, sqs, start=True, stop=True)
                absd = rows.tile((1, T), FP, tag="absd", bufs=2)
                act(absd, pden, Act.Abs)
                dabs = rows.tile((1, T), FP, tag="dabs", bufs=2)
                vec.tensor_scalar(dabs, absd, 1.0, None, Alu.max)
                rden = rows.tile((1, T), FP, tag="rden", bufs=2)
                vec.reciprocal(rden, dabs)
                meanh = rows.tile((1, T), FP, tag="meanh", bufs=2)
                vec.scalar_tensor_tensor(meanh, pmu, 1.0 / DH, rden, Alu.mult, Alu.mult)
                rden2 = rows.tile((1, T), FP, tag="rden2", bufs=2)
                act(rden2, rden, Act.Square)
                ex2a = rows.tile((1, T), FP, tag="ex2a", bufs=2)
                vec.scalar_tensor_tensor(ex2a, psq, 1.0 / DH, rden2, Alu.mult, Alu.mult)
                msqh = rows.tile((1, T), FP, tag="msqh", bufs=2)
                act(msqh, meanh, Act.Square)
                varh = rows.tile((1, T), FP, tag="varh", bufs=2)
                vec.tensor_tensor(varh, ex2a, msqh, Alu.subtract)
                sdh = rows.tile((1, T), FP, tag="sdh", bufs=2)
                act(sdh, varh, Act.Sqrt, bias=epsb)
                rsh = rows.tile((1, T), FP, tag="rsh", bufs=2)
                vec.reciprocal(rsh, sdh)
                Arow = rows.tile((1, T), FP, tag="Arow", bufs=2)
                vec.tensor_tensor(Arow, rden, rsh, Alu.mult)
                negm2 = rows.tile((1, T), FP, tag="two2", bufs=2)
                vec.scalar_tensor_tensor(negm2, meanh, -1.0, rsh,
                                         Alu.mult, Alu.mult)
                pB = pp.tile((128, T), FP, tag="pb", bufs=2)
                mm(pB, ggs[h], negm2, start=True, stop=False)
                mm(pB, ggb[h], ones1T, start=False, stop=True)
                pA = pp.tile((128, T), FP, tag="pb", bufs=2)
                mm(pA, onesrow, Arow, start=True, stop=True)
                t1 = acts.tile((128, T), FP, tag="t1", bufs=2)
                vec.tensor_tensor(t1, nums, pA, Alu.mult)
                t2 = acts.tile((128, T), FP, tag="t2", bufs=2)
                vec.scalar_tensor_tensor(t2, t1, gnc[:, h:h + 1], pB, Alu.mult, Alu.add)
                ot = acts.tile((128, T), FP, tag=f"oT{h}", bufs=2)
                vec.tensor_tensor(_r(ot), t2, sz[h], Alu.mult)
                outT.append(ot)

            # ---- Phase F: W_down + residual ----
            newh = []
            for dc in range(DC):
                pd = pp.tile((128, T), FP, tag="pb", bufs=2)
                for kc in range(NKC):
                    wdt = stream.tile((128, 128), FP, tag="wqk", bufs=6)
                    dma(out=_r(wdt),
                        in_=_r(d["wdown"][l * I + kc * 128:l * I + (kc + 1) * 128,
                                          dc * 128:(dc + 1) * 128]))
                    mm(pd, _r(wdt), _r(outT[kc]), start=(kc == 0), stop=(kc == NKC - 1))
                nh = acts.tile((128, T), FP, tag=f"hT{dc}", bufs=2)
                vec.tensor_tensor(nh, hT[dc], pd, Alu.add)
                newh.append(nh)
            hT = newh

        # ---- final LN on last timestep + MDN logits ----
        pss = pp.tile((1, 1), FP, tag="prow", bufs=2)
        for dc in range(DC):
            mm(pss, hT[dc][:, T - 1:T], onescol, start=(dc == 0), stop=(dc == DC - 1))
        psq2 = pp.tile((1, 1), FP, tag="prow", bufs=2)
        for dc in range(DC):
            mm(psq2, hT[dc][:, T - 1:T], hT[dc][:, T - 1:T],
               start=(dc == 0), stop=(dc == DC - 1))
        mean1 = rows.tile((1, 1), FP, tag="s1", bufs=1)
        vec.tensor_scalar_mul(mean1, pss, 1.0 / D)
        ex21 = rows.tile((1, 1), FP, tag="s2", bufs=1)
        vec.tensor_scalar_mul(ex21, psq2, 1.0 / D)
        msq1 = rows.tile((1, 1), FP, tag="s3", bufs=1)
        act(msq1, mean1, Act.Square)
        var1 = rows.tile((1, 1), FP, tag="s4", bufs=1)
        vec.tensor_tensor(var1, ex21, msq1, Alu.subtract)
        std1 = rows.tile((1, 1), FP, tag="s5", bufs=1)
        act(std1, var1, Act.Sqrt, bias=epsb)
        rstd1 = rows.tile((1, 1), FP, tag="s6", bufs=1)
        vec.reciprocal(rstd1, std1)
        two3 = rows.tile((1, 2), FP, tag="s7", bufs=1)
        vec.tensor_copy(two3[:, 0:1], rstd1)
        vec.scalar_tensor_tensor(two3[:, 1:2], mean1, -1.0, rstd1, Alu.mult, Alu.mult)
        pbc = pp.tile((128, 2), FP, tag="prow", bufs=2)
        mm(pbc, onesrow, two3, start=True, stop=True)
        bc = acts.tile((128, 2), FP, tag="bc", bufs=1)
        act(bc, pbc, Act.Copy)
        lastn = []
        for dc in range(DC):
            ta = acts.tile((128, 1), FP, tag=f"la{dc}", bufs=1)
            vec.scalar_tensor_tensor(ta, hT[dc][:, T - 1:T], bc[:, 0:1], bc[:, 1:2],
                                     Alu.mult, Alu.add)
            lt = acts.tile((128, 1), FP, tag=f"ln{dc}", bufs=1)
            vec.scalar_tensor_tensor(_r(lt), ta, flns[:, dc:dc + 1], flnb[:, dc:dc + 1],
                                     Alu.mult, Alu.add)
            lastn.append(lt)
        for j in range(9):
            n = 512 if j < 8 else MT - 8 * 512
            pms = pp.tile((1, n), FP, tag="pvg", bufs=2)
            for dc in range(DC):
                wct = stream.tile((128, n), FP, tag="wv", bufs=3)
                if n >= 256:
                    dma(out=_r(wct),
                        in_=_r(d["wcat"][dc * 128:(dc + 1) * 128,
                                         j * 512:j * 512 + n]))
                else:
                    dma(out=wct,
                        in_=d["wcat"][dc * 128:(dc + 1) * 128, j * 512:j * 512 + n])
                if n >= 256:
                    mm(pms, _r(lastn[dc]), _r(wct), start=(dc == 0), stop=(dc == DC - 1))
                else:
                    mm(pms, lastn[dc], wct, start=(dc == 0), stop=(dc == DC - 1))
            mrow = rows.tile((1, n), FP, tag="mrow", bufs=3)
            act(mrow, pms, Act.Copy)
            dma(out=d["mall"][0:1, j * 512:j * 512 + n], in_=mrow)

    nc.finalize()
    return nc


_CACHE = {}
_LAST = {"exec_ns": None}


def _get_nc():
    if "nc" not in _CACHE:
        _CACHE["nc"] = build_nc()
    return _CACHE["nc"]


def kernel(**inputs):
    nc = _get_nc()
    f = np.float32

    def g(k):
        return np.asarray(inputs[k], f)

    x = g("x")
    wproj = g("W_proj").reshape(L * D, 2 * I)
    wq = g("Wq").reshape(L * I, I)
    wk = g("Wk").reshape(L * I, I)
    wv = g("Wv").reshape(L * I, I)
    wi = g("Wi").reshape(L * I, H)
    wf = g("Wf").reshape(L * I, H)
    bib = g("bi").reshape(L * H, 1)
    bfn = (-g("bf")).reshape(L * H, 1)
    ln_s, ln_b = g("ln_s"), g("ln_b")
    lgr = np.zeros((L * 2 * DC, 128), f)
    lnsc = np.zeros((L * 128, DC), f)
    for l in range(L):
        for dc in range(DC):
            lgr[l * 2 * DC + 2 * dc] = ln_s[l, dc * 128:(dc + 1) * 128]
            lgr[l * 2 * DC + 2 * dc + 1] = ln_b[l, dc * 128:(dc + 1) * 128]
        lnsc[l * 128:(l + 1) * 128, :] = ln_s[l].reshape(DC, 128).T
    gn_s, gn_b = g("gn_s"), g("gn_b")
    ggr = np.zeros((L * 2 * H, 128), f)
    gnc = np.zeros((L * 128, H), f)
    for l in range(L):
        for h in range(H):
            ggr[l * 2 * H + 2 * h] = gn_s[l, h * 128:(h + 1) * 128]
            ggr[l * 2 * H + 2 * h + 1] = gn_b[l, h * 128:(h + 1) * 128]
        gnc[l * 128:(l + 1) * 128, :] = gn_s[l].reshape(H, 128).T
    wdown = g("W_down").reshape(L * I, D)
    tg = np.arange(T)[None, :]
    sg = np.arange(128)[:, None]
    ltb0 = np.where(sg <= tg, 0.0, NEG).astype(f)
    ltb1 = np.where(sg + 128 <= tg, 0.0, NEG).astype(f)
    ident = np.eye(128, dtype=f)
    rsel = np.kron(np.eye(H, dtype=f), np.ones((1, 128), f))
    flns = g("fln_s").reshape(DC, 128).T.copy()
    flnb = g("fln_b").reshape(DC, 128).T.copy()
    wcat = np.concatenate([g("W_pi"), g("W_mu"), g("W_sig")], axis=1)

    shared = dict(wproj=wproj, wq=wq, wk=wk, wv=wv, wi=wi, wf=wf, bib=bib,
                  bfn=bfn, lgr=lgr, lnsc=lnsc, ggr=ggr, gnc=gnc, wdown=wdown,
                  ltb0=ltb0, ltb1=ltb1, ident=ident, rsel=rsel, flns=flns,
                  flnb=flnb, wcat=wcat)
    shared = {k: np.ascontiguousarray(v) for k, v in shared.items()}
    in_maps = []
    for c in range(B):
        m = dict(shared)
        m["xT"] = np.ascontiguousarray(x[c].T)
        in_maps.append(m)

    trace = os.environ.get("KERNEL_TRACE") == "1"
    res = bass_utils.run_bass_kernel_spmd(nc, in_maps, core_ids=list(range(B)),
                                          trace=trace)
    _LAST["exec_ns"] = res.exec_time_ns
    mall = np.stack([np.asarray(res.results[c]["mall"]).reshape(MT)
                     for c in range(B)]).astype(np.float64)

    b_pi = np.asarray(inputs["b_pi"], np.float64)
    b_mu = np.asarray(inputs["b_mu"], np.float64)
    b_sig = np.asarray(inputs["b_sig"], np.float64)
    lp = mall[:, :K] + b_pi
    e = np.exp(lp - lp.max(-1, keepdims=True))
    pi = (e / e.sum(-1, keepdims=True)).astype(f)
    mu = (mall[:, K:K + K * G] + b_mu).reshape(B, K, G).astype(f)
    sigma = np.logaddexp(0.0, mall[:, K + K * G:] + b_sig).reshape(B, K, G).astype(f)
    return pi, mu, sigma
